# revision 38
# baseline (speedup 1.0000x reference)
"""CharRNN Trainium2 kernel: data-parallel over batch on 8 NeuronCores.

kernel(**inputs) takes the FULL unsharded inputs (as produced by
setup_inputs) and returns the full [128, 1024, 128] float32 logits.
Each core runs 16 batch rows through the full T=1024 tanh recurrence.

Numerics (bf16 hi/lo pairs everywhere -> rel_inf ~3e-3 vs f32 ref):
  W_hh ~ W_hi + W_lo (bf16), h ~ h_hi + h_lo (bf16, h_hi = bf16 tanh,
  h_lo = bf16(f32 tanh - h_hi)), xp ~ xp_hi + xp_lo.  3 products per
  step: W_lo@h_hi (early pass, N=16) and W_hi@[h_hi|h_lo] (merged
  moving, N=32, stride-0 psum broadcast).  W_lo@h_lo ~ 2^-24, dropped.
  fc reads BOTH h planes (exact), f32 psum/bias.

Perf design (PE pair cadence ~27-32ns; psum->ACT handshake ~340ns,
ACT ~314ns, so the serial tanh tail dominates):
  - ONE psum group [128, (jt,b)=64] per step: inject + 16 W_lo MMs
    (gated only by h_hi = the bf16 tanh) + 16 merged W_hi MMs (gated
    by h_lo = f32 tanh - h_hi, ~750ns later).  The early pass plus
    fc/xp fillers execute under the h_lo-production shadow.
  - tail per step: ACT1 tanh->bf16 h_hi, ACT2 tanh->f32 hT, DVE sub
    -> h_lo.  fc needs no repack: the h ring [128, it, hi/lo, S, B]
    keeps each (tile, plane) [slot, b] block contiguous, so fc block
    matmuls use the ring directly as stationary (the baseline's
    1.3-1.5us strided DVE repacks and the head-of-line PE stall they
    caused are gone).
  - xp_t = (mp_hi + mp_lo)[x_t] built per 32-step chunk via onehot
    matmuls; injected by one identity matmul (N=128, hi/lo broadcast-
    accumulated).  All fillers are emitted after the step's matmuls.
"""
import numpy as np

import concourse.bacc as bacc
import concourse.mybir as mybir
from concourse.tile import TileContext
from concourse.masks import make_identity

f32 = mybir.dt.float32
bf16 = mybir.dt.bfloat16

B = 16        # batch rows per core
H = 512
NT = 4        # hidden tiles
V = 128
E = 16
CH = 32       # steps per xp chunk
FC = 8        # steps per fc block
S = 32        # h ring slots (32 so fc reads never WAR-block the tanh ACT)
AF = mybir.ActivationFunctionType
ALU = mybir.AluOpType


def build(T: int = 1024):
    assert T % CH == 0 and CH % FC == 0 and S % FC == 0
    nc = bacc.Bacc("TRN2", target_bir_lowering=False, debug=False)

    x_tb = nc.declare_dram_parameter("x_tb", [T, B], f32, isOutput=False)
    emb = nc.declare_dram_parameter("emb", [V, E], f32, isOutput=False)
    W_ih = nc.declare_dram_parameter("W_ih", [H, E], f32, isOutput=False)
    W_hh = nc.declare_dram_parameter("W_hh", [H, H], f32, isOutput=False)
    bias = nc.declare_dram_parameter("bias", [1, H], f32, isOutput=False)  # b_ih+b_hh
    W_fc = nc.declare_dram_parameter("W_fc", [V, H], f32, isOutput=False)
    b_fc = nc.declare_dram_parameter("b_fc", [1, V], f32, isOutput=False)
    out = nc.declare_dram_parameter("out", [B, T, V], f32, isOutput=True)

    out_t = out.rearrange("b t v -> t b v")
    n_chunks = T // CH

    with TileContext(nc) as tc:
        with (
            tc.tile_pool(name="const", bufs=1) as cpool,
            tc.tile_pool(name="hs", bufs=1) as hspool,
            tc.tile_pool(name="xp", bufs=1) as xppool,
            tc.tile_pool(name="work", bufs=2) as wkpool,
            tc.tile_pool(name="ps_rec", bufs=2, space="PSUM") as ps_rec,
            tc.tile_pool(name="ps_xp", bufs=2, space="PSUM") as ps_xp,
            tc.tile_pool(name="ps_fc", bufs=2, space="PSUM") as ps_fc,
        ):
            # ---------------- one-time prep ----------------
            ident_f32 = cpool.tile([128, 128], f32, tag="ident")
            make_identity(nc, ident_f32)
            ident16 = cpool.tile([128, 128], bf16, tag="identh")
            nc.vector.tensor_copy(ident16[:, :], ident_f32[:, :])

            # W_hhT tiles [128_k, it, jt*128] bf16 hi/lo via PE transpose
            w_nat = wkpool.tile([128, NT, H], f32, tag="wnat")  # [j_p, jt, i]
            nc.sync.dma_start(
                w_nat[:, :, :], W_hh.rearrange("(jt p) i -> p jt i", p=128)
            )
            whh_hi = cpool.tile([128, NT, H], bf16, tag="whh_hi")
            whh_lo = cpool.tile([128, NT, H], bf16, tag="whh_lo")
            for it in range(NT):
                for jt in range(NT):
                    tp = ps_xp.tile([128, 128], f32, tag="xpp")
                    nc.tensor.transpose(
                        tp[:, :],
                        w_nat[:, jt, it * 128 : (it + 1) * 128],
                        ident_f32[:, :],
                    )
                    nc.vector.tensor_copy(
                        whh_hi[:, it, jt * 128 : (jt + 1) * 128], tp[:, :]
                    )
                    nc.vector.tensor_tensor(
                        whh_lo[:, it, jt * 128 : (jt + 1) * 128],
                        tp[:, :],
                        whh_hi[:, it, jt * 128 : (jt + 1) * 128],
                        ALU.subtract,
                    )

            # mp = emb @ W_ih.T + bias -> [128_v, H] bf16 hi/lo
            embT = wkpool.tile([E, V], f32, tag="embT")
            nc.sync.dma_start(embT[:, :], emb.rearrange("v e -> e v"))
            wihT = wkpool.tile([E, H], f32, tag="wihT")
            nc.sync.dma_start(wihT[:, :], W_ih.rearrange("h e -> e h"))
            mp_ps = ps_xp.tile([128, H], f32, tag="xpp")
            nc.tensor.matmul(mp_ps[:, :], embT[:, :], wihT[:, :], start=True, stop=True)
            bias_row = wkpool.tile([1, H], f32, tag="biasrow")
            nc.sync.dma_start(bias_row[:, :], bias[:, :])
            bias_bc = wkpool.tile([128, H], f32, tag="biasbc")
            nc.gpsimd.partition_broadcast(bias_bc[:, :], bias_row[:, :])
            mp_f = wkpool.tile([128, H], f32, tag="mpf")
            nc.vector.tensor_tensor(mp_f[:, :], mp_ps[:, :], bias_bc[:, :], ALU.add)
            mp_hi = cpool.tile([128, H], bf16, tag="mp_hi")
            mp_lo = cpool.tile([128, H], bf16, tag="mp_lo")
            nc.vector.tensor_copy(mp_hi[:, :], mp_f[:, :])
            nc.vector.tensor_tensor(mp_lo[:, :], mp_f[:, :], mp_hi[:, :], ALU.subtract)

            # W_fcT tiles [128_j, jt, 128_v] bf16 via PE transpose
            wfc_nat = wkpool.tile([128, H], f32, tag="wfcnat")  # [v_p, j]
            nc.sync.dma_start(wfc_nat[:, :], W_fc[:, :])
            wfcT = cpool.tile([128, NT, V], bf16, tag="wfcT")
            for jt in range(NT):
                tp = ps_xp.tile([128, 128], f32, tag="xpp")
                nc.tensor.transpose(
                    tp[:, :], wfc_nat[:, jt * 128 : (jt + 1) * 128], ident_f32[:, :]
                )
                nc.vector.tensor_copy(wfcT[:, jt, :], tp[:, :])

            # b_fc broadcast [128_tok, V]
            bfc_row = wkpool.tile([1, V], f32, tag="bfcrow")
            nc.sync.dma_start(bfc_row[:, :], b_fc[:, :])
            bfc_bc = cpool.tile([128, V], f32, tag="bfcbc")
            nc.gpsimd.partition_broadcast(bfc_bc[:, :], bfc_row[:, :])

            # iota column [128, 1] for onehot compares
            iota_col = cpool.tile([128, 1], f32, tag="iota")
            nc.gpsimd.iota(iota_col[:, :], pattern=[[0, 1]], channel_multiplier=1,
                           allow_small_or_imprecise_dtypes=True)

            # h ring [128, it, {hi,lo}, S, B] bf16: per (tile, plane) the
            # [slot, b] block is contiguous -> fc stationary needs no repack.
            hs = hspool.tile([128, NT, 2, S, B], bf16, tag="hs")
            for it in range(NT):
                nc.vector.memset(hs[:, it, :, S - 1, :], 0.0)

            # xp chunk double buffers [128_j, CH, {hi,lo}, NT, B] bf16
            xp_bufs = [
                xppool.tile([128, CH, 2, NT, B], bf16, tag=f"xp{p}", name=f"xp{p}")
                for p in range(2)
            ]

            # ---------------- xp chunk machinery ----------------
            onehot_cur = [None]

            xbc_cur = [None]

            def xp_prep(c):
                xrow = wkpool.tile([1, CH * B], f32, tag="xrow")
                nc.sync.dma_start(
                    xrow[:, :],
                    x_tb.rearrange("(a t) b -> a (t b)", t=CH)[c : c + 1, :],
                )
                xbc = wkpool.tile([128, CH * B], f32, tag="xbc")
                nc.gpsimd.partition_broadcast(xbc[:, :], xrow[:, :])
                xbc_cur[0] = xbc
                onehot_cur[0] = wkpool.tile(
                    [128, CH * B], bf16, tag="onehot", name="onehot"
                )

            def xp_onehot(c, piece):
                # halves (~270ns DVE) so the op never blocks a critical sub
                cols = slice(piece * CH * B // 2, (piece + 1) * CH * B // 2)
                nc.vector.tensor_scalar(
                    onehot_cur[0][:, cols], xbc_cur[0][:, cols], iota_col[:, :],
                    None, ALU.is_equal,
                )

            xp_pend = {}

            def xp_mm(c, k):
                jt, g = k // 2, k % 2
                mp = mp_hi if g == 0 else mp_lo
                ps = ps_xp.tile([128, CH * B], f32, tag="xpp")
                nc.tensor.matmul(
                    ps[:, :], mp[:, jt * 128 : (jt + 1) * 128],
                    onehot_cur[0][:, :], start=True, stop=True,
                )
                xp_pend[k] = ps

            def xp_scatter(c, k, piece):
                # halves (~350ns DVE) for the same reason as xp_onehot
                jt, g = k // 2, k % 2
                ps = xp_pend[k] if piece == 0 else xp_pend.pop(k)
                sl = slice(piece * CH // 2, (piece + 1) * CH // 2)
                nc.vector.tensor_copy(
                    xp_bufs[c % 2][:, sl, g, jt, :],
                    ps.rearrange("p (s b) -> p s b", s=CH)[:, sl, :],
                )

            # ---------------- fc machinery ----------------
            fc_pend = {}
            fc_lg = {}

            def fc_mm(b0, jt):
                fslot = (b0 * FC) % S
                if jt == 0:
                    ps = ps_fc.tile([128, V], f32, tag="fcp")
                    fc_pend[b0] = ps
                else:
                    ps = fc_pend[b0]
                for g in (0, 1):
                    first = jt == 0 and g == 0
                    last = jt == 3 and g == 1
                    nc.tensor.matmul(
                        ps[:, :],
                        hs[:, jt, g, fslot : fslot + FC, :].rearrange(
                            "p a b -> p (a b)"
                        ),
                        wfcT[:, jt, :],
                        start=first, stop=last,
                        skip_group_check=not (first or last),
                    )

            def fc_bias(b0):
                ps = fc_pend.pop(b0)
                lg = wkpool.tile([128, V], f32, tag="logits")
                nc.vector.tensor_tensor(lg[:, :], ps[:, :], bfc_bc[:, :], ALU.add)
                fc_lg[b0] = lg

            def fc_dma(b0):
                lg = fc_lg.pop(b0)
                nc.sync.dma_start(out_t[b0 * FC : (b0 + 1) * FC, :, :], lg[:, :])

            # ---------------- recurrence ----------------
            def rec_step(s):
                c, si = s // CH, s % CH
                slot, pslot = s % S, (s - 1) % S
                par = c % 2
                psr = ps_rec.tile([128, NT * B], f32, tag="rec")

                # inject xp hi+lo (N=128, stride-0 accumulate over planes)
                o = psr.rearrange("p (o c) -> p o c", o=1).broadcast_to(
                    [128, 2, NT * B]
                )
                nc.tensor.matmul(
                    o, ident16[:, :],
                    xp_bufs[par][:, si, :, :, :].rearrange("p a b c -> p (a b c)"),
                    start=True, stop=False, skip_group_check=True,
                )
                # early pass: W_lo @ h_hi  (needs only the bf16 tanh)
                for it in range(NT):
                    m = hs[:, it, 0, pslot, :]
                    for jt in range(NT):
                        nc.tensor.matmul(
                            psr[:, jt * B : (jt + 1) * B],
                            whh_lo[:, it, jt * 128 : (jt + 1) * 128],
                            m,
                            start=False, stop=False, skip_group_check=True,
                        )
                # late pass: W_hi @ [h_hi|h_lo] merged (stride-0 accumulate)
                for it in range(NT):
                    m = hs[:, it, :, pslot, :]
                    for jt in range(NT):
                        last = it == NT - 1 and jt == NT - 1
                        ob = psr[:, jt * B : (jt + 1) * B].rearrange(
                            "p (o b) -> p o b", o=1
                        ).broadcast_to([128, 2, B])
                        nc.tensor.matmul(
                            ob,
                            whh_hi[:, it, jt * 128 : (jt + 1) * 128],
                            m,
                            start=False, stop=last,
                            skip_group_check=not last,
                        )
                # tail: h_hi = bf16 tanh; hT = f32 tanh; h_lo = hT - h_hi.
                # The sub runs in halves so the next step's Whi it=2,3 matmuls
                # wait only on subB (8 post-h_lo matmuls instead of 16).
                nc.scalar.activation(
                    hs[:, :, 0, slot, :],
                    psr.rearrange("p (a b) -> p a b", a=NT), AF.Tanh,
                )
                hT = wkpool.tile([128, NT * B], f32, tag="hT")
                nc.scalar.activation(hT[:, :], psr[:, :], AF.Tanh)
                hT4 = hT.rearrange("p (a b) -> p a b", a=NT)
                nc.vector.tensor_tensor(
                    hs[:, 0:2, 1, slot, :],
                    hT4[:, 0:2, :],
                    hs[:, 0:2, 0, slot, :],
                    ALU.subtract,
                )
                nc.vector.tensor_tensor(
                    hs[:, 2:4, 1, slot, :],
                    hT4[:, 2:4, :],
                    hs[:, 2:4, 0, slot, :],
                    ALU.subtract,
                )

            def fillers(s):
                c, si = s // CH, s % CH
                # prefetch chunk c+2's x row / broadcast a whole chunk early
                # so the onehot and first xp matmul never stall at a chunk
                # boundary (the DMA + gpsimd broadcast take ~2 steps).
                if si == 24 and c + 2 < n_chunks:
                    xp_prep(c + 2)
                if c + 1 < n_chunks:
                    # PE: mm_k at even si 6..20.  DVE: scatter_k halves at
                    # si 8+2k, 9+2k (done before mm_{k+2} reuses the psum
                    # bank at si 10+2k, bufs=2).
                    if si in (1, 2):
                        xp_onehot(c + 1, si - 1)
                    elif si >= 6 and si % 2 == 0 and si < 22:
                        xp_mm(c + 1, (si - 6) // 2)
                    if 8 <= si < 24:
                        xp_scatter(c + 1, (si - 8) // 2, si % 2)
                # fc: one jt piece (2 MMs, ~210ns) per odd step so a filler
                # burst never exceeds the tanh-shadow window.
                k = s % FC
                if s >= FC and k % 2 == 1:
                    fc_mm(s // FC - 1, k // 2)
                if s >= 2 * FC:
                    if k == 0:
                        fc_bias(s // FC - 2)
                    elif k == 2:
                        fc_dma(s // FC - 2)

            # ---------------- main schedule ----------------
            xp_prep(0)
            xp_onehot(0, 0)
            xp_onehot(0, 1)
            for k in range(2 * NT):
                xp_mm(0, k)
                xp_scatter(0, k, 0)
                xp_scatter(0, k, 1)
            xp_prep(1)  # chunk 1's row/broadcast before its onehot at s=1,2

            for s in range(T):
                rec_step(s)
                fillers(s)

            for jt in range(NT):
                fc_mm(T // FC - 1, jt)
            for b0 in (T // FC - 2, T // FC - 1):
                fc_bias(b0)
                fc_dma(b0)

    nc.finalize()
    return nc


_NC_CACHE = {}


def kernel(x, emb, W_ih, W_hh, b_ih, b_hh, W_fc, b_fc):
    from concourse.bass_utils import run_bass_kernel_spmd

    T_full = 1024
    x = np.asarray(x)
    emb = np.asarray(emb, dtype=np.float32)
    W_ih = np.asarray(W_ih, dtype=np.float32)
    W_hh = np.asarray(W_hh, dtype=np.float32)
    b_ih = np.asarray(b_ih, dtype=np.float32)
    b_hh = np.asarray(b_hh, dtype=np.float32)
    W_fc = np.asarray(W_fc, dtype=np.float32)
    b_fc = np.asarray(b_fc, dtype=np.float32)

    if "nc" not in _NC_CACHE:
        _NC_CACHE["nc"] = build(T_full)
    nc = _NC_CACHE["nc"]

    bias = (b_ih + b_hh).reshape(1, H).astype(np.float32)
    in_maps = []
    for core in range(8):
        xs = x[core * B : (core + 1) * B, :]          # [16, 1024]
        in_maps.append(dict(
            x_tb=np.ascontiguousarray(xs.T).astype(np.float32),
            emb=emb, W_ih=W_ih, W_hh=W_hh, bias=bias,
            W_fc=W_fc, b_fc=b_fc.reshape(1, V),
        ))
    res = run_bass_kernel_spmd(nc, in_maps, core_ids=list(range(8)))
    return np.concatenate([r["out"] for r in res.results], axis=0)
